# revision 2
# baseline (speedup 1.0000x reference)
"""Cross-attention + FFN + layernorm block on 8 Trainium2 NeuronCores, v2.

Sharding: data-parallel over (B=4) x (LQ split in 2) -> 8 shards of 1024
query rows; keys/values/weights replicated per batch pair, no collectives.

v2 design (vs the PE-transpose/f32r baseline):
  - Everything bf16 on device; host converts inputs + weights (free).
  - All transposes via the DMA crossbar (dma_start transpose=True): x^T
    loads straight from DRAM, o^T / att^T SBUF->SBUF. Zero PE/DVE cost.
  - One resident [128, C, 768] copy of each weight, loaded once.
  - Attention: scoresT[k,q] per (head, kc) for both q-halves sharing one
    lhsT load; exp is split across engines: Act does true exp (bias =
    -50 mask), DVE/Pool compute a bf16 fast-exp via the int bit trick
    (uint16 = round(x*128/ln2 + 16250.5 + mask)); attn@[v|1] accumulates
    o rows + softmax denominator in one PSUM tile per q-half.
  - Emission interleaves k/q projections chunk-wise with attention head
    pairs so exp latency hides behind projection matmuls.
"""

import sys

if '/opt/trn_rl_repo' not in sys.path:
    sys.path.insert(0, '/opt/trn_rl_repo')

import numpy as np
import ml_dtypes

B, LQ, LK, D, H = 4, 2048, 2048, 768, 12
DH = D // H            # 64
NC = 8                 # cores
LQC = B * LQ // NC     # 1024 query rows per core
QB = LQC // 128        # 8 q row-tiles
KT = LK // 128         # 16 k row-tiles
C = D // 128           # 6 feature chunks
EPS = 1e-5
S16 = float(np.float32(128.0 / np.log(2.0)))   # fast-exp scale
C16 = 16256.0 - 5.5                            # fast-exp magic bias
MASK_B = -50.0                                 # additive mask bias

_CACHE = {}


def _build(debug=False):
    import concourse.bacc as bacc
    import concourse.bass as bass
    import concourse.tile as tile
    import concourse.mybir as mybir
    from concourse.masks import make_identity

    f32 = mybir.dt.float32
    bf16 = mybir.dt.bfloat16
    u16 = mybir.dt.uint16
    Exp = mybir.ActivationFunctionType.Exp
    Relu = mybir.ActivationFunctionType.Relu
    Sqrt = mybir.ActivationFunctionType.Sqrt
    Copy = mybir.ActivationFunctionType.Copy
    Add = mybir.AluOpType.add
    Mult = mybir.AluOpType.mult
    Sub = mybir.AluOpType.subtract

    nc = bacc.Bacc("TRN2", target_bir_lowering=False, debug=False)

    # host passes x pre-transposed: [128(d within chunk), C(d chunk), L]
    xq = nc.dram_tensor("xq", [128, C, LQC], bf16, kind="ExternalInput")
    xk = nc.dram_tensor("xk", [128, C, LK], bf16, kind="ExternalInput")
    xv = nc.dram_tensor("xv", [128, C, LK], bf16, kind="ExternalInput")
    mba = nc.dram_tensor("mba", [128, KT], f32, kind="ExternalInput")
    mbv = nc.dram_tensor("mbv", [128, KT], f32, kind="ExternalInput")
    wq = nc.dram_tensor("wq", [D, D], bf16, kind="ExternalInput")
    wk = nc.dram_tensor("wk", [D, D], bf16, kind="ExternalInput")
    wv = nc.dram_tensor("wv", [D, D], bf16, kind="ExternalInput")
    wo = nc.dram_tensor("wo", [D, D], bf16, kind="ExternalInput")
    w1 = nc.dram_tensor("w1", [D, D], bf16, kind="ExternalInput")
    w2 = nc.dram_tensor("w2", [D, D], bf16, kind="ExternalInput")
    b1c = nc.dram_tensor("b1c", [128, C], f32, kind="ExternalInput")
    b2v = nc.dram_tensor("b2v", [1, D], bf16, kind="ExternalInput")
    gv = nc.dram_tensor("gv", [D], f32, kind="ExternalInput")
    bv = nc.dram_tensor("bv", [D], f32, kind="ExternalInput")
    yout = nc.dram_tensor("yout", [LQC, D], f32, kind="ExternalOutput")
    if debug:
        dbg = {
            name: nc.dram_tensor(name, shape, mybir.dt.bfloat16,
                                 kind="ExternalOutput")
            for name, shape in [
                ("d_qT", [128, C, LQC]), ("d_kT", [128, C, LK]),
                ("d_vp", [128, KT, H, DH + 1]), ("d_osb", [128, QB, D]),
                ("d_oT", [128, C, LQC]), ("d_attT", [128, C, LQC]),
                ("d_attrm", [128, QB, D]), ("d_hT", [128, C, LQC]),
                ("d_xqT", [128, C, LQC]),
            ]}

    def wrow_ap(w):
        # [128(din part), C(din chunk), D(dout)] view of a [D, D] weight
        return w.ap().rearrange("(c p) n -> p c n", p=128)

    def bcast_ap(v):
        a = v.ap()
        return bass.AP(tensor=a.tensor, offset=a.offset, ap=[[0, 128]] + list(a.ap))

    with tile.TileContext(nc) as tc:
        with tc.tile_pool(name="consts", bufs=1) as consts, \
             tc.tile_pool(name="persist", bufs=1) as persist, \
             tc.tile_pool(name="work", bufs=2) as work, \
             tc.tile_pool(name="expool", bufs=6) as expool, \
             tc.tile_pool(name="pp_big", bufs=6, space="PSUM") as pp_big, \
             tc.tile_pool(name="pp_pos", bufs=1, space="PSUM") as pp_pos:

            # ---- persistent tensors (tags = disjoint-lifetime slot shares)
            wv_t = persist.tile([128, C, D], bf16, tag="wvw1")
            wk_t = persist.tile([128, C, D], bf16, tag="wkw2")
            wq_t = persist.tile([128, C, D], bf16, tag="wqwo")
            xvT = persist.tile([128, C, LK], bf16, tag="xvkT")
            xkT = persist.tile([128, C, LK], bf16, tag="xkrm")
            xqT = persist.tile([128, C, LQC], bf16, tag="xqatT")
            vp = persist.tile([128, KT, H, DH + 1], bf16, tag="vp")
            qT = persist.tile([128, C, LQC], bf16, tag="qThT")
            o_sb = persist.tile([128, QB, D], bf16, tag="osb")
            oT = persist.tile([128, C, LQC], bf16, tag="oT")

            # ---- input loads (all SWDGE): host pre-transposed x, quartered
            # so the v/k projections start on the first k-quarter. The only
            # xbar-transpose users left (oT, att_rm) all sit on the SP ring.
            nc.gpsimd.dma_start(out=wv_t, in_=wrow_ap(wv))
            for j in range(4):
                nc.gpsimd.dma_start(out=xvT[:, :, j * 512:(j + 1) * 512],
                                    in_=xv.ap()[:, :, j * 512:(j + 1) * 512])
            nc.gpsimd.dma_start(out=wk_t, in_=wrow_ap(wk))
            for j in range(4):
                nc.gpsimd.dma_start(out=xkT[:, :, j * 512:(j + 1) * 512],
                                    in_=xk.ap()[:, :, j * 512:(j + 1) * 512])
            nc.gpsimd.dma_start(out=wq_t, in_=wrow_ap(wq))
            for j in range(2):
                nc.gpsimd.dma_start(out=xqT[:, :, j * 512:(j + 1) * 512],
                                    in_=xq.ap()[:, :, j * 512:(j + 1) * 512])

            # ---- constants (SWDGE; emitted after the critical-path loads)
            mba_t = consts.tile([128, KT], f32)
            nc.gpsimd.dma_start(out=mba_t, in_=mba.ap())
            mbv_t = consts.tile([128, KT], f32)
            nc.gpsimd.dma_start(out=mbv_t, in_=mbv.ap())
            b1_t = consts.tile([128, C], f32)
            nc.gpsimd.dma_start(out=b1_t, in_=b1c.ap())
            b2r_t = consts.tile([1, D], bf16)
            nc.gpsimd.dma_start(out=b2r_t, in_=b2v.ap())
            one_t = consts.tile([1, 128], bf16)
            nc.vector.memset(one_t, 1.0)
            ident = consts.tile([128, 128], bf16)
            make_identity(nc, ident)
            g_t = consts.tile([128, D], f32)
            nc.gpsimd.dma_start(out=g_t, in_=bcast_ap(gv))
            be_t = consts.tile([128, D], f32)
            nc.gpsimd.dma_start(out=be_t, in_=bcast_ap(bv))
            eps_t = consts.tile([128, 1], f32)
            nc.vector.memset(eps_t, EPS)

            # ---- v projection: vp[k, t, h, 0:64] row-major + ones column
            for t in range(KT):
                ps5 = pp_big.tile([128, 512], f32, tag="p512", name=f"vps5_{t}")
                ps2 = pp_big.tile([128, 512], f32, tag="p512", name=f"vps2_{t}")
                for c in range(C):
                    nc.tensor.matmul(ps5[:], xvT[:, c, t * 128:(t + 1) * 128],
                                     wv_t[:, c, 0:512],
                                     start=(c == 0), stop=(c == C - 1))
                    nc.tensor.matmul(ps2[:, 0:256], xvT[:, c, t * 128:(t + 1) * 128],
                                     wv_t[:, c, 512:768],
                                     start=(c == 0), stop=(c == C - 1))
                # Pool can't touch PSUM on TRN2: copies go DVE / Act(Copy)
                if t % 2 == 0:
                    nc.vector.tensor_copy(
                        out=vp[:, t, 0:8, 0:DH],
                        in_=ps5[:].rearrange("p (h d) -> p h d", d=DH))
                    nc.scalar.activation(
                        out=vp[:, t, 8:12, 0:DH],
                        in_=ps2[:, 0:256].rearrange("p (h d) -> p h d", d=DH),
                        func=Copy)
                else:
                    nc.scalar.activation(
                        out=vp[:, t, 0:8, 0:DH],
                        in_=ps5[:].rearrange("p (h d) -> p h d", d=DH),
                        func=Copy)
                    nc.vector.tensor_copy(
                        out=vp[:, t, 8:12, 0:DH],
                        in_=ps2[:, 0:256].rearrange("p (h d) -> p h d", d=DH))
            nc.vector.memset(vp[:, :, :, DH:DH + 1], 1.0)

            # kT lives in xvT's slot (xvT dead once the v projection ends)
            kT = persist.tile([128, C, LK], bf16, tag="xvkT", name="kT")

            # ---- exp engine picker: Act=true exp (4/7), DVE=fast-exp (3/7)
            def emit_exp(ps, ex, h, kc, qc):
                if (2 * kc + qc) % 7 < 4:
                    nc.scalar.activation(out=ex[:], in_=ps[:], func=Exp,
                                         bias=mba_t[:, kc:kc + 1], scale=1.0)
                else:
                    nc.vector.tensor_scalar(
                        out=ex.bitcast(u16), in0=ps[:],
                        scalar1=S16, scalar2=mbv_t[:, kc:kc + 1],
                        op0=Mult, op1=Add)

            def attention(h):
                p0 = (h % 2) * 64
                cc = h // 2
                # one q-half at a time; two accumulators share each PSUM
                # bank: the bank-mate at +128 runs start=False always, its
                # first accumulate lands on the bank zero from qs0's start
                for qc in range(2):
                    pos = [pp_pos.tile([128, 256], f32, tag=f"po{b}",
                                       name=f"po_{h}_{qc}_{b}")
                           for b in range(2)]

                    def pslot(qs):
                        return pos[qs // 2][:, (qs % 2) * 128:(qs % 2) * 128 + 65]

                    exq = {}
                    PD = 5  # attnV runs PD score-tiles behind: hides exp
                    for kc in range(KT + PD):
                        if kc < KT:
                            ps_s = pp_big.tile([128, 512], f32, tag="p512",
                                               name=f"sc_{h}_{kc}_{qc}")
                            nc.tensor.matmul(
                                ps_s[:],
                                kT[p0:p0 + 64, cc, kc * 128:(kc + 1) * 128],
                                qT[p0:p0 + 64, cc, qc * 512:(qc + 1) * 512],
                                start=True, stop=True)
                            ex = expool.tile([128, 512], bf16, tag="ex",
                                             name=f"ex_{h}_{kc}_{qc}")
                            emit_exp(ps_s, ex, h, kc, qc)
                            exq[kc] = ex
                        if kc >= PD:
                            k0 = kc - PD
                            ex = exq.pop(k0)
                            for qs in range(4):
                                nc.tensor.matmul(
                                    pslot(qs),
                                    ex[:, qs * 128:(qs + 1) * 128],
                                    vp[:, k0, h, :],
                                    start=(k0 == 0 and qs % 2 == 0),
                                    stop=(k0 == KT - 1),
                                    skip_group_check=True)
                    rec = work.tile([128, 4], f32, tag="rec",
                                    name=f"rec_{h}_{qc}")
                    for qs in range(4):
                        nc.vector.reciprocal(rec[:, qs:qs + 1],
                                             pslot(qs)[:, DH:DH + 1])
                    for qs in range(4):
                        if qs % 2 == 0:
                            nc.scalar.activation(
                                out=o_sb[:, qc * 4 + qs, h * DH:(h + 1) * DH],
                                in_=pslot(qs)[:, 0:DH], func=Copy,
                                scale=rec[:, qs:qs + 1])
                        else:
                            nc.vector.tensor_scalar_mul(
                                out=o_sb[:, qc * 4 + qs, h * DH:(h + 1) * DH],
                                in0=pslot(qs)[:, 0:DH],
                                scalar1=rec[:, qs:qs + 1])

            # ---- interleaved: k/q projection chunk n, then heads 2n, 2n+1
            for n in range(C):
                psk = [pp_big.tile([128, 512], f32, tag="p512",
                                   name=f"kp_{n}_{j}") for j in range(4)]
                for c in range(C):
                    for j in range(4):
                        nc.tensor.matmul(
                            psk[j][:], wk_t[:, c, n * 128:(n + 1) * 128],
                            xkT[:, c, j * 512:(j + 1) * 512],
                            start=(c == 0), stop=(c == C - 1))
                for j in range(4):
                    if j % 2 == 0:
                        nc.vector.tensor_copy(
                            out=kT[:, n, j * 512:(j + 1) * 512], in_=psk[j][:])
                    else:
                        nc.scalar.activation(
                            out=kT[:, n, j * 512:(j + 1) * 512], in_=psk[j][:],
                            func=Copy)
                psq = [pp_big.tile([128, 512], f32, tag="p512",
                                   name=f"qp_{n}_{j}") for j in range(2)]
                for c in range(C):
                    for j in range(2):
                        nc.tensor.matmul(
                            psq[j][:], wq_t[:, c, n * 128:(n + 1) * 128],
                            xqT[:, c, j * 512:(j + 1) * 512],
                            start=(c == 0), stop=(c == C - 1))
                nc.vector.tensor_copy(out=qT[:, n, 0:512], in_=psq[0][:])
                nc.scalar.activation(out=qT[:, n, 512:1024], in_=psq[1][:],
                                     func=Copy)

                attention(2 * n)
                attention(2 * n + 1)
                # o^T chunk n ready: transpose out (DMA xbar, SP queue)
                for qb in range(QB):
                    nc.sync.dma_start(
                        out=oT[:, n, qb * 128:(qb + 1) * 128],
                        in_=o_sb[:, qb, n * 128:(n + 1) * 128],
                        transpose=True)

            if debug:
                for name, t in [("d_qT", qT), ("d_kT", kT), ("d_vp", vp),
                                ("d_osb", o_sb), ("d_xqT", xqT)]:
                    nc.sync.dma_start(out=dbg[name].ap(), in_=t[:])

            # ---- att^T = Wo^T @ oT ; att row-major via DMA transpose
            wo_t = persist.tile([128, C, D], bf16, tag="wqwo", name="wo_t")
            nc.gpsimd.dma_start(out=wo_t, in_=wrow_ap(wo))
            w1_t = persist.tile([128, C, D], bf16, tag="wvw1", name="w1_t")
            nc.gpsimd.dma_start(out=w1_t, in_=wrow_ap(w1))
            w2_t = persist.tile([128, C, D], bf16, tag="wkw2", name="w2_t")
            nc.gpsimd.dma_start(out=w2_t, in_=wrow_ap(w2))

            attT = persist.tile([128, C, LQC], bf16, tag="xqatT", name="attT")
            att_rm = persist.tile([128, QB, D], bf16, tag="xkrm", name="att_rm")
            for n in range(C):
                psa = [pp_big.tile([128, 512], f32, tag="p512",
                                   name=f"at_{n}_{j}") for j in range(2)]
                for c in range(C):
                    for j in range(2):
                        nc.tensor.matmul(
                            psa[j][:], wo_t[:, c, n * 128:(n + 1) * 128],
                            oT[:, c, j * 512:(j + 1) * 512],
                            start=(c == 0), stop=(c == C - 1))
                nc.vector.tensor_copy(out=attT[:, n, 0:512], in_=psa[0][:])
                nc.scalar.activation(out=attT[:, n, 512:1024], in_=psa[1][:],
                                     func=Copy)
                for qb in range(QB):
                    nc.sync.dma_start(
                        out=att_rm[:, qb, n * 128:(n + 1) * 128],
                        in_=attT[:, n, qb * 128:(qb + 1) * 128],
                        transpose=True)

            # ---- h^T = relu(W1^T @ attT + b1)
            hT = persist.tile([128, C, LQC], bf16, tag="qThT", name="hT")
            for n in range(C):
                psh = [pp_big.tile([128, 512], f32, tag="p512",
                                   name=f"h_{n}_{j}") for j in range(2)]
                for c in range(C):
                    for j in range(2):
                        nc.tensor.matmul(
                            psh[j][:], w1_t[:, c, n * 128:(n + 1) * 128],
                            attT[:, c, j * 512:(j + 1) * 512],
                            start=(c == 0), stop=(c == C - 1))
                nc.scalar.activation(
                    out=hT[:, n, 0:512], in_=psh[0][:],
                    func=Relu, bias=b1_t[:, n:n + 1], scale=1.0)
                nc.vector.tensor_scalar(
                    out=hT[:, n, 512:1024], in0=psh[1][:],
                    scalar1=b1_t[:, n:n + 1], scalar2=0.0,
                    op0=Add, op1=mybir.AluOpType.max)

            # ---- ffn out + residual + layernorm per q row-tile
            for qb in range(QB):
                ps5 = pp_big.tile([128, 512], f32, tag="p512", name=f"f5_{qb}")
                ps2 = pp_big.tile([128, 512], f32, tag="p512", name=f"f2_{qb}")
                for c in range(C):
                    nc.tensor.matmul(ps5[:], hT[:, c, qb * 128:(qb + 1) * 128],
                                     w2_t[:, c, 0:512],
                                     start=(c == 0), stop=False)
                    nc.tensor.matmul(ps2[:, 0:256],
                                     hT[:, c, qb * 128:(qb + 1) * 128],
                                     w2_t[:, c, 512:768],
                                     start=(c == 0), stop=False)
                # + b2 (ones-column bcast) and + att residual (identity) on PE
                nc.tensor.matmul(ps5[:], one_t[:], b2r_t[:, 0:512],
                                 start=False, stop=False)
                nc.tensor.matmul(ps2[:, 0:256], one_t[:], b2r_t[:, 512:768],
                                 start=False, stop=False)
                nc.tensor.matmul(ps5[:], ident[:], att_rm[:, qb, 0:512],
                                 start=False, stop=True)
                nc.tensor.matmul(ps2[:, 0:256], ident[:],
                                 att_rm[:, qb, 512:768],
                                 start=False, stop=True)
                y = work.tile([128, D], f32, tag="y", name=f"y_{qb}")
                nc.scalar.activation(out=y[:, 0:512], in_=ps5[:], func=Copy)
                nc.scalar.activation(out=y[:, 512:768], in_=ps2[:, 0:256],
                                     func=Copy)
                stats = work.tile([128, 3, 6], f32, tag="stats",
                                  name=f"st_{qb}")
                for sg in range(3):
                    nc.vector.bn_stats(out=stats[:, sg, :],
                                       in_=y[:, sg * 256:(sg + 1) * 256])
                mv = work.tile([128, 2], f32, tag="mv", name=f"mv_{qb}")
                nc.vector.bn_aggr(out=mv[:], in_=stats[:])
                rstd = work.tile([128, 1], f32, tag="rstd", name=f"rs_{qb}")
                nc.scalar.activation(out=rstd[:], in_=mv[:, 1:2], func=Sqrt,
                                     bias=eps_t[:], scale=1.0)
                nc.vector.reciprocal(rstd[:], rstd[:])
                yn = work.tile([128, D], f32, tag="yn", name=f"yn_{qb}")
                nc.vector.scalar_tensor_tensor(
                    out=yn[:], in0=y[:], scalar=mv[:, 0:1], in1=g_t[:],
                    op0=Sub, op1=Mult)
                nc.vector.scalar_tensor_tensor(
                    out=yn[:], in0=yn[:], scalar=rstd[:], in1=be_t[:],
                    op0=Mult, op1=Add)
                nc.sync.dma_start(out=yout.ap()[qb * 128:(qb + 1) * 128, :],
                                  in_=yn[:])

            if debug:
                for name, t in [("d_oT", oT), ("d_attT", attT),
                                ("d_attrm", att_rm), ("d_hT", hT)]:
                    nc.sync.dma_start(out=dbg[name].ap(), in_=t[:])

    nc.compile()
    return nc


def _get_nc():
    if "nc" not in _CACHE:
        _CACHE["nc"] = _build(debug=_CACHE.get("debug", False))
    return _CACHE["nc"]


def _prepare_in_maps(queries, keys, values, mask, Wq, Wk, Wv, Wo, W1, b1,
                     W2, b2, ln_g, ln_b):
    bf = ml_dtypes.bfloat16
    queries = np.asarray(queries, np.float32).astype(bf)
    keys = np.asarray(keys, np.float32).astype(bf)
    values = np.asarray(values, np.float32).astype(bf)
    mask = np.asarray(mask)

    valid = (mask != 0).sum(axis=1).astype(np.int64)        # [B]
    kidx = np.arange(LK)
    masked = (kidx[None, :] >= valid[:, None])              # [B, LK]
    # per-batch [128, KT] bias planes: index = kc*128 + p
    mb_a = np.where(masked, MASK_B, 0.0).astype(np.float32)
    mb_a = mb_a.reshape(B, KT, 128).transpose(0, 2, 1).copy()
    mb_v = np.where(masked, C16 + MASK_B * S16, C16).astype(np.float32)
    mb_v = mb_v.reshape(B, KT, 128).transpose(0, 2, 1).copy()

    wq_s = (np.asarray(Wq, np.float32) / np.sqrt(np.float32(DH))).astype(bf)
    common = {
        "wq": wq_s,
        "wk": np.asarray(Wk, np.float32).astype(bf),
        "wv": np.asarray(Wv, np.float32).astype(bf),
        "wo": np.asarray(Wo, np.float32).astype(bf),
        "w1": np.asarray(W1, np.float32).astype(bf),
        "w2": np.asarray(W2, np.float32).astype(bf),
        "b1c": np.ascontiguousarray(
            np.asarray(b1, np.float32).reshape(C, 128).T),
        "b2v": np.asarray(b2, np.float32).astype(bf).reshape(1, D),
        "gv": np.ascontiguousarray(ln_g, np.float32),
        "bv": np.ascontiguousarray(ln_b, np.float32),
    }

    def tmajor(x):
        # [L, D] -> [128, C, L] feature-major chunks
        return np.ascontiguousarray(x.T.reshape(C, 128, -1).transpose(1, 0, 2))

    in_maps = []
    kv_t = [tmajor(keys[b]) for b in range(B)]
    vv_t = [tmajor(values[b]) for b in range(B)]
    for core in range(NC):
        b, half = core // 2, core % 2
        in_maps.append(dict(
            common,
            xq=tmajor(queries[b, half * LQC:(half + 1) * LQC, :]),
            xk=kv_t[b],
            xv=vv_t[b],
            mba=np.ascontiguousarray(mb_a[b]),
            mbv=np.ascontiguousarray(mb_v[b]),
        ))
    return in_maps


def kernel(queries, keys, values, mask, Wq, Wk, Wv, Wo, W1, b1, W2, b2,
           ln_g, ln_b, _trace=False):
    from concourse.bass_utils import run_bass_kernel_spmd

    in_maps = _prepare_in_maps(queries, keys, values, mask, Wq, Wk, Wv, Wo,
                               W1, b1, W2, b2, ln_g, ln_b)
    nc = _get_nc()
    res = run_bass_kernel_spmd(nc, in_maps, core_ids=list(range(NC)),
                               trace=_trace)
    _CACHE["last_result"] = res

    out = np.empty((B, LQ, D), dtype=np.float32)
    for core in range(NC):
        b, half = core // 2, core % 2
        out[b, half * LQC:(half + 1) * LQC, :] = res.results[core]["yout"]
    return out


# revision 3
# speedup vs baseline: 1.0029x; 1.0029x over previous
"""Cross-attention + FFN + layernorm block on 8 Trainium2 NeuronCores, v2.

Sharding: data-parallel over (B=4) x (LQ split in 2) -> 8 shards of 1024
query rows; keys/values/weights replicated per batch pair, no collectives.

v2 design (vs the PE-transpose/f32r baseline):
  - Everything bf16 on device; host converts inputs + weights (free).
  - All transposes via the DMA crossbar (dma_start transpose=True): x^T
    loads straight from DRAM, o^T / att^T SBUF->SBUF. Zero PE/DVE cost.
  - One resident [128, C, 768] copy of each weight, loaded once.
  - Attention: scoresT[k,q] per (head, kc) for both q-halves sharing one
    lhsT load; exp is split across engines: Act does true exp (bias =
    -50 mask), DVE/Pool compute a bf16 fast-exp via the int bit trick
    (uint16 = round(x*128/ln2 + 16250.5 + mask)); attn@[v|1] accumulates
    o rows + softmax denominator in one PSUM tile per q-half.
  - Emission interleaves k/q projections chunk-wise with attention head
    pairs so exp latency hides behind projection matmuls.
"""

import sys

if '/opt/trn_rl_repo' not in sys.path:
    sys.path.insert(0, '/opt/trn_rl_repo')

import numpy as np
import ml_dtypes

B, LQ, LK, D, H = 4, 2048, 2048, 768, 12
DH = D // H            # 64
NC = 8                 # cores
LQC = B * LQ // NC     # 1024 query rows per core
QB = LQC // 128        # 8 q row-tiles
KT = LK // 128         # 16 k row-tiles
C = D // 128           # 6 feature chunks
EPS = 1e-5
S16 = float(np.float32(128.0 / np.log(2.0)))   # fast-exp scale
C16 = 16256.0 - 5.5                            # fast-exp magic bias
MASK_B = -50.0                                 # additive mask bias

_CACHE = {}


def _build(debug=False):
    import concourse.bacc as bacc
    import concourse.bass as bass
    import concourse.tile as tile
    import concourse.mybir as mybir
    from concourse.masks import make_identity

    f32 = mybir.dt.float32
    bf16 = mybir.dt.bfloat16
    u16 = mybir.dt.uint16
    Exp = mybir.ActivationFunctionType.Exp
    Relu = mybir.ActivationFunctionType.Relu
    Sqrt = mybir.ActivationFunctionType.Sqrt
    Copy = mybir.ActivationFunctionType.Copy
    Add = mybir.AluOpType.add
    Mult = mybir.AluOpType.mult
    Sub = mybir.AluOpType.subtract

    nc = bacc.Bacc("TRN2", target_bir_lowering=False, debug=False)

    # host passes x pre-transposed: [128(d within chunk), C(d chunk), L]
    xq = nc.dram_tensor("xq", [128, C, LQC], bf16, kind="ExternalInput")
    xk = nc.dram_tensor("xk", [128, C, LK], bf16, kind="ExternalInput")
    xv = nc.dram_tensor("xv", [128, C, LK], bf16, kind="ExternalInput")
    mba = nc.dram_tensor("mba", [128, KT], f32, kind="ExternalInput")
    mbv = nc.dram_tensor("mbv", [128, KT], f32, kind="ExternalInput")
    wq = nc.dram_tensor("wq", [D, D], bf16, kind="ExternalInput")
    wk = nc.dram_tensor("wk", [D, D], bf16, kind="ExternalInput")
    wv = nc.dram_tensor("wv", [D, D], bf16, kind="ExternalInput")
    wo = nc.dram_tensor("wo", [D, D], bf16, kind="ExternalInput")
    w1 = nc.dram_tensor("w1", [D, D], bf16, kind="ExternalInput")
    w2 = nc.dram_tensor("w2", [D, D], bf16, kind="ExternalInput")
    b1c = nc.dram_tensor("b1c", [128, C], f32, kind="ExternalInput")
    b2v = nc.dram_tensor("b2v", [1, D], bf16, kind="ExternalInput")
    gv = nc.dram_tensor("gv", [D], f32, kind="ExternalInput")
    bv = nc.dram_tensor("bv", [D], f32, kind="ExternalInput")
    yout = nc.dram_tensor("yout", [LQC, D], f32, kind="ExternalOutput")
    if debug:
        dbg = {
            name: nc.dram_tensor(name, shape, mybir.dt.bfloat16,
                                 kind="ExternalOutput")
            for name, shape in [
                ("d_qT", [128, C, LQC]), ("d_kT", [128, C, LK]),
                ("d_vp", [128, KT, H, DH + 1]), ("d_osb", [128, QB, D]),
                ("d_oT", [128, C, LQC]), ("d_attT", [128, C, LQC]),
                ("d_attrm", [128, QB, D]), ("d_hT", [128, C, LQC]),
                ("d_xqT", [128, C, LQC]),
            ]}

    def wrow_ap(w):
        # [128(din part), C(din chunk), D(dout)] view of a [D, D] weight
        return w.ap().rearrange("(c p) n -> p c n", p=128)

    def bcast_ap(v):
        a = v.ap()
        return bass.AP(tensor=a.tensor, offset=a.offset, ap=[[0, 128]] + list(a.ap))

    with tile.TileContext(nc) as tc:
        with tc.tile_pool(name="consts", bufs=1) as consts, \
             tc.tile_pool(name="persist", bufs=1) as persist, \
             tc.tile_pool(name="work", bufs=2) as work, \
             tc.tile_pool(name="expool", bufs=6) as expool, \
             tc.tile_pool(name="pp_big", bufs=6, space="PSUM") as pp_big, \
             tc.tile_pool(name="pp_pos", bufs=1, space="PSUM") as pp_pos:

            # ---- persistent tensors (tags = disjoint-lifetime slot shares)
            wv_t = persist.tile([128, C, D], bf16, tag="wvw1")
            wk_t = persist.tile([128, C, D], bf16, tag="wkw2")
            wq_t = persist.tile([128, C, D], bf16, tag="wqwo")
            xvT = persist.tile([128, C, LK], bf16, tag="xvkT")
            xkT = persist.tile([128, C, LK], bf16, tag="xkrm")
            xqT = persist.tile([128, C, LQC], bf16, tag="xqatT")
            vp = persist.tile([128, KT, H, DH + 1], bf16, tag="vp")
            qT = persist.tile([128, C, LQC], bf16, tag="qThT")
            o_sb = persist.tile([128, QB, D], bf16, tag="osb")
            oT = persist.tile([128, C, LQC], bf16, tag="oT")

            # ---- input loads (all SWDGE): host pre-transposed x, quartered
            # so the v/k projections start on the first k-quarter. The only
            # xbar-transpose users left (oT, att_rm) all sit on the SP ring.
            nc.gpsimd.dma_start(out=wv_t, in_=wrow_ap(wv))
            for j in range(4):
                nc.gpsimd.dma_start(out=xvT[:, :, j * 512:(j + 1) * 512],
                                    in_=xv.ap()[:, :, j * 512:(j + 1) * 512])
            nc.gpsimd.dma_start(out=wk_t, in_=wrow_ap(wk))
            for j in range(4):
                nc.gpsimd.dma_start(out=xkT[:, :, j * 512:(j + 1) * 512],
                                    in_=xk.ap()[:, :, j * 512:(j + 1) * 512])
            nc.gpsimd.dma_start(out=wq_t, in_=wrow_ap(wq))
            for j in range(2):
                nc.gpsimd.dma_start(out=xqT[:, :, j * 512:(j + 1) * 512],
                                    in_=xq.ap()[:, :, j * 512:(j + 1) * 512])

            # ---- constants (SWDGE; emitted after the critical-path loads)
            mba_t = consts.tile([128, KT], f32)
            nc.gpsimd.dma_start(out=mba_t, in_=mba.ap())
            mbv_t = consts.tile([128, KT], f32)
            nc.gpsimd.dma_start(out=mbv_t, in_=mbv.ap())
            b1_t = consts.tile([128, C], f32)
            nc.gpsimd.dma_start(out=b1_t, in_=b1c.ap())
            b2r_t = consts.tile([1, D], bf16)
            nc.gpsimd.dma_start(out=b2r_t, in_=b2v.ap())
            one_t = consts.tile([1, 128], bf16)
            nc.vector.memset(one_t, 1.0)
            ident = consts.tile([128, 128], bf16)
            make_identity(nc, ident)
            g_t = consts.tile([128, D], f32)
            nc.gpsimd.dma_start(out=g_t, in_=bcast_ap(gv))
            be_t = consts.tile([128, D], f32)
            nc.gpsimd.dma_start(out=be_t, in_=bcast_ap(bv))
            eps_t = consts.tile([128, 1], f32)
            nc.vector.memset(eps_t, EPS)

            # ---- v projection: vp[k, t, h, 0:64] row-major + ones column
            for t in range(KT):
                ps5 = pp_big.tile([128, 512], f32, tag="p512", name=f"vps5_{t}")
                ps2 = pp_big.tile([128, 512], f32, tag="p512", name=f"vps2_{t}")
                for c in range(C):
                    nc.tensor.matmul(ps5[:], xvT[:, c, t * 128:(t + 1) * 128],
                                     wv_t[:, c, 0:512],
                                     start=(c == 0), stop=(c == C - 1))
                    nc.tensor.matmul(ps2[:, 0:256], xvT[:, c, t * 128:(t + 1) * 128],
                                     wv_t[:, c, 512:768],
                                     start=(c == 0), stop=(c == C - 1))
                # Pool can't touch PSUM on TRN2: copies go DVE / Act(Copy)
                if t % 2 == 0:
                    nc.vector.tensor_copy(
                        out=vp[:, t, 0:8, 0:DH],
                        in_=ps5[:].rearrange("p (h d) -> p h d", d=DH))
                    nc.scalar.activation(
                        out=vp[:, t, 8:12, 0:DH],
                        in_=ps2[:, 0:256].rearrange("p (h d) -> p h d", d=DH),
                        func=Copy)
                else:
                    nc.scalar.activation(
                        out=vp[:, t, 0:8, 0:DH],
                        in_=ps5[:].rearrange("p (h d) -> p h d", d=DH),
                        func=Copy)
                    nc.vector.tensor_copy(
                        out=vp[:, t, 8:12, 0:DH],
                        in_=ps2[:, 0:256].rearrange("p (h d) -> p h d", d=DH))
            nc.vector.memset(vp[:, :, :, DH:DH + 1], 1.0)

            # kT lives in xvT's slot (xvT dead once the v projection ends)
            kT = persist.tile([128, C, LK], bf16, tag="xvkT", name="kT")

            # ---- exp engine picker: Act=true exp (every row gets 5/8 of
            # its tiles as true exp), DVE=fast-exp for the rest
            def emit_exp(ps, ex, h, kc, qc):
                if (2 * kc + qc) % 7 < 4:
                    nc.scalar.activation(out=ex[:], in_=ps[:], func=Exp,
                                         bias=mba_t[:, kc:kc + 1], scale=1.0)
                else:
                    nc.vector.tensor_scalar(
                        out=ex.bitcast(u16), in0=ps[:],
                        scalar1=S16, scalar2=mbv_t[:, kc:kc + 1],
                        op0=Mult, op1=Add)

            def attention(h):
                p0 = (h % 2) * 64
                cc = h // 2
                # one q-half at a time; two accumulators share each PSUM
                # bank: the bank-mate at +128 runs start=False always, its
                # first accumulate lands on the bank zero from qs0's start
                for qc in range(2):
                    pos = [pp_pos.tile([128, 256], f32, tag=f"po{b}",
                                       name=f"po_{h}_{qc}_{b}")
                           for b in range(2)]

                    def pslot(qs):
                        return pos[qs // 2][:, (qs % 2) * 128:(qs % 2) * 128 + 65]

                    exq = {}
                    PD = 5  # attnV runs PD score-tiles behind: hides exp
                    for kc in range(KT + PD):
                        if kc < KT:
                            ps_s = pp_big.tile([128, 512], f32, tag="p512",
                                               name=f"sc_{h}_{kc}_{qc}")
                            nc.tensor.matmul(
                                ps_s[:],
                                kT[p0:p0 + 64, cc, kc * 128:(kc + 1) * 128],
                                qT[p0:p0 + 64, cc, qc * 512:(qc + 1) * 512],
                                start=True, stop=True)
                            ex = expool.tile([128, 512], bf16, tag="ex",
                                             name=f"ex_{h}_{kc}_{qc}")
                            emit_exp(ps_s, ex, h, kc, qc)
                            exq[kc] = ex
                        if kc >= PD:
                            k0 = kc - PD
                            ex = exq.pop(k0)
                            for qs in range(4):
                                nc.tensor.matmul(
                                    pslot(qs),
                                    ex[:, qs * 128:(qs + 1) * 128],
                                    vp[:, k0, h, :],
                                    start=(k0 == 0 and qs % 2 == 0),
                                    stop=(k0 == KT - 1),
                                    skip_group_check=True)
                    rec = work.tile([128, 4], f32, tag="rec",
                                    name=f"rec_{h}_{qc}")
                    for b in range(2):
                        nc.vector.reciprocal(
                            rec[:, b * 2:(b + 1) * 2],
                            pos[b].rearrange("p (g s) -> p g s",
                                             s=128)[:, :, DH])
                    for b in range(2):
                        # normalize both bank-groups in one op: in1 is the
                        # per-group reciprocal broadcast along the free dim
                        rap = rec[:, b * 2:(b + 1) * 2]
                        bc = bass.AP(tensor=rap.tensor, offset=rap.offset,
                                     ap=[list(rap.ap[0]), list(rap.ap[1]),
                                         [0, DH]])
                        nc.vector.scalar_tensor_tensor(
                            out=o_sb[:, qc * 4 + 2 * b:qc * 4 + 2 * b + 2,
                                     h * DH:(h + 1) * DH],
                            in0=pos[b].rearrange("p (g s) -> p g s",
                                                 s=128)[:, :, 0:DH],
                            scalar=1.0, op0=Mult,
                            in1=bc, op1=Mult)

            # ---- interleaved: k/q projection chunk n, then heads 2n, 2n+1
            for n in range(C):
                psk = [pp_big.tile([128, 512], f32, tag="p512",
                                   name=f"kp_{n}_{j}") for j in range(4)]
                for c in range(C):
                    for j in range(4):
                        nc.tensor.matmul(
                            psk[j][:], wk_t[:, c, n * 128:(n + 1) * 128],
                            xkT[:, c, j * 512:(j + 1) * 512],
                            start=(c == 0), stop=(c == C - 1))
                for j in range(4):
                    if j % 2 == 0:
                        nc.vector.tensor_copy(
                            out=kT[:, n, j * 512:(j + 1) * 512], in_=psk[j][:])
                    else:
                        nc.scalar.activation(
                            out=kT[:, n, j * 512:(j + 1) * 512], in_=psk[j][:],
                            func=Copy)
                psq = [pp_big.tile([128, 512], f32, tag="p512",
                                   name=f"qp_{n}_{j}") for j in range(2)]
                for c in range(C):
                    for j in range(2):
                        nc.tensor.matmul(
                            psq[j][:], wq_t[:, c, n * 128:(n + 1) * 128],
                            xqT[:, c, j * 512:(j + 1) * 512],
                            start=(c == 0), stop=(c == C - 1))
                nc.vector.tensor_copy(out=qT[:, n, 0:512], in_=psq[0][:])
                nc.scalar.activation(out=qT[:, n, 512:1024], in_=psq[1][:],
                                     func=Copy)

                attention(2 * n)
                attention(2 * n + 1)
                # o^T chunk n ready: transpose out (DMA xbar, SP queue)
                for qb in range(QB):
                    nc.sync.dma_start(
                        out=oT[:, n, qb * 128:(qb + 1) * 128],
                        in_=o_sb[:, qb, n * 128:(n + 1) * 128],
                        transpose=True)

            if debug:
                for name, t in [("d_qT", qT), ("d_kT", kT), ("d_vp", vp),
                                ("d_osb", o_sb), ("d_xqT", xqT)]:
                    nc.sync.dma_start(out=dbg[name].ap(), in_=t[:])

            # ---- att^T = Wo^T @ oT ; att row-major via DMA transpose
            wo_t = persist.tile([128, C, D], bf16, tag="wqwo", name="wo_t")
            nc.gpsimd.dma_start(out=wo_t, in_=wrow_ap(wo))
            w1_t = persist.tile([128, C, D], bf16, tag="wvw1", name="w1_t")
            nc.gpsimd.dma_start(out=w1_t, in_=wrow_ap(w1))
            w2_t = persist.tile([128, C, D], bf16, tag="wkw2", name="w2_t")
            nc.gpsimd.dma_start(out=w2_t, in_=wrow_ap(w2))

            attT = persist.tile([128, C, LQC], bf16, tag="xqatT", name="attT")
            att_rm = persist.tile([128, QB, D], bf16, tag="xkrm", name="att_rm")
            for n in range(C):
                psa = [pp_big.tile([128, 512], f32, tag="p512",
                                   name=f"at_{n}_{j}") for j in range(2)]
                for c in range(C):
                    for j in range(2):
                        nc.tensor.matmul(
                            psa[j][:], wo_t[:, c, n * 128:(n + 1) * 128],
                            oT[:, c, j * 512:(j + 1) * 512],
                            start=(c == 0), stop=(c == C - 1))
                nc.vector.tensor_copy(out=attT[:, n, 0:512], in_=psa[0][:])
                nc.scalar.activation(out=attT[:, n, 512:1024], in_=psa[1][:],
                                     func=Copy)
                for qb in range(QB):
                    nc.sync.dma_start(
                        out=att_rm[:, qb, n * 128:(n + 1) * 128],
                        in_=attT[:, n, qb * 128:(qb + 1) * 128],
                        transpose=True)

            # ---- h^T = relu(W1^T @ attT + b1)
            hT = persist.tile([128, C, LQC], bf16, tag="qThT", name="hT")
            for n in range(C):
                psh = [pp_big.tile([128, 512], f32, tag="p512",
                                   name=f"h_{n}_{j}") for j in range(2)]
                for c in range(C):
                    for j in range(2):
                        nc.tensor.matmul(
                            psh[j][:], w1_t[:, c, n * 128:(n + 1) * 128],
                            attT[:, c, j * 512:(j + 1) * 512],
                            start=(c == 0), stop=(c == C - 1))
                nc.scalar.activation(
                    out=hT[:, n, 0:512], in_=psh[0][:],
                    func=Relu, bias=b1_t[:, n:n + 1], scale=1.0)
                nc.vector.tensor_scalar(
                    out=hT[:, n, 512:1024], in0=psh[1][:],
                    scalar1=b1_t[:, n:n + 1], scalar2=0.0,
                    op0=Add, op1=mybir.AluOpType.max)

            # ---- ffn out + residual + layernorm per q row-tile
            for qb in range(QB):
                ps5 = pp_big.tile([128, 512], f32, tag="p512", name=f"f5_{qb}")
                ps2 = pp_big.tile([128, 512], f32, tag="p512", name=f"f2_{qb}")
                for c in range(C):
                    nc.tensor.matmul(ps5[:], hT[:, c, qb * 128:(qb + 1) * 128],
                                     w2_t[:, c, 0:512],
                                     start=(c == 0), stop=False)
                    nc.tensor.matmul(ps2[:, 0:256],
                                     hT[:, c, qb * 128:(qb + 1) * 128],
                                     w2_t[:, c, 512:768],
                                     start=(c == 0), stop=False)
                # + b2 (ones-column bcast) and + att residual (identity) on PE
                nc.tensor.matmul(ps5[:], one_t[:], b2r_t[:, 0:512],
                                 start=False, stop=False)
                nc.tensor.matmul(ps2[:, 0:256], one_t[:], b2r_t[:, 512:768],
                                 start=False, stop=False)
                nc.tensor.matmul(ps5[:], ident[:], att_rm[:, qb, 0:512],
                                 start=False, stop=True)
                nc.tensor.matmul(ps2[:, 0:256], ident[:],
                                 att_rm[:, qb, 512:768],
                                 start=False, stop=True)
                y = work.tile([128, D], f32, tag="y", name=f"y_{qb}")
                nc.scalar.activation(out=y[:, 0:512], in_=ps5[:], func=Copy)
                nc.scalar.activation(out=y[:, 512:768], in_=ps2[:, 0:256],
                                     func=Copy)
                stats = work.tile([128, 3, 6], f32, tag="stats",
                                  name=f"st_{qb}")
                for sg in range(3):
                    nc.vector.bn_stats(out=stats[:, sg, :],
                                       in_=y[:, sg * 256:(sg + 1) * 256])
                mv = work.tile([128, 2], f32, tag="mv", name=f"mv_{qb}")
                nc.vector.bn_aggr(out=mv[:], in_=stats[:])
                rstd = work.tile([128, 1], f32, tag="rstd", name=f"rs_{qb}")
                nc.scalar.activation(out=rstd[:], in_=mv[:, 1:2], func=Sqrt,
                                     bias=eps_t[:], scale=1.0)
                nc.vector.reciprocal(rstd[:], rstd[:])
                yn = work.tile([128, D], f32, tag="yn", name=f"yn_{qb}")
                nc.vector.scalar_tensor_tensor(
                    out=yn[:], in0=y[:], scalar=mv[:, 0:1], in1=g_t[:],
                    op0=Sub, op1=Mult)
                nc.vector.scalar_tensor_tensor(
                    out=yn[:], in0=yn[:], scalar=rstd[:], in1=be_t[:],
                    op0=Mult, op1=Add)
                nc.sync.dma_start(out=yout.ap()[qb * 128:(qb + 1) * 128, :],
                                  in_=yn[:])

            if debug:
                for name, t in [("d_oT", oT), ("d_attT", attT),
                                ("d_attrm", att_rm), ("d_hT", hT)]:
                    nc.sync.dma_start(out=dbg[name].ap(), in_=t[:])

    nc.compile()
    return nc


def _get_nc():
    if "nc" not in _CACHE:
        _CACHE["nc"] = _build(debug=_CACHE.get("debug", False))
    return _CACHE["nc"]


def _prepare_in_maps(queries, keys, values, mask, Wq, Wk, Wv, Wo, W1, b1,
                     W2, b2, ln_g, ln_b):
    bf = ml_dtypes.bfloat16
    queries = np.asarray(queries, np.float32).astype(bf)
    keys = np.asarray(keys, np.float32).astype(bf)
    values = np.asarray(values, np.float32).astype(bf)
    mask = np.asarray(mask)

    valid = (mask != 0).sum(axis=1).astype(np.int64)        # [B]
    kidx = np.arange(LK)
    masked = (kidx[None, :] >= valid[:, None])              # [B, LK]
    # per-batch [128, KT] bias planes: index = kc*128 + p
    mb_a = np.where(masked, MASK_B, 0.0).astype(np.float32)
    mb_a = mb_a.reshape(B, KT, 128).transpose(0, 2, 1).copy()
    mb_v = np.where(masked, C16 + MASK_B * S16, C16).astype(np.float32)
    mb_v = mb_v.reshape(B, KT, 128).transpose(0, 2, 1).copy()

    wq_s = (np.asarray(Wq, np.float32) / np.sqrt(np.float32(DH))).astype(bf)
    common = {
        "wq": wq_s,
        "wk": np.asarray(Wk, np.float32).astype(bf),
        "wv": np.asarray(Wv, np.float32).astype(bf),
        "wo": np.asarray(Wo, np.float32).astype(bf),
        "w1": np.asarray(W1, np.float32).astype(bf),
        "w2": np.asarray(W2, np.float32).astype(bf),
        "b1c": np.ascontiguousarray(
            np.asarray(b1, np.float32).reshape(C, 128).T),
        "b2v": np.asarray(b2, np.float32).astype(bf).reshape(1, D),
        "gv": np.ascontiguousarray(ln_g, np.float32),
        "bv": np.ascontiguousarray(ln_b, np.float32),
    }

    def tmajor(x):
        # [L, D] -> [128, C, L] feature-major chunks
        return np.ascontiguousarray(x.T.reshape(C, 128, -1).transpose(1, 0, 2))

    in_maps = []
    kv_t = [tmajor(keys[b]) for b in range(B)]
    vv_t = [tmajor(values[b]) for b in range(B)]
    for core in range(NC):
        b, half = core // 2, core % 2
        in_maps.append(dict(
            common,
            xq=tmajor(queries[b, half * LQC:(half + 1) * LQC, :]),
            xk=kv_t[b],
            xv=vv_t[b],
            mba=np.ascontiguousarray(mb_a[b]),
            mbv=np.ascontiguousarray(mb_v[b]),
        ))
    return in_maps


def kernel(queries, keys, values, mask, Wq, Wk, Wv, Wo, W1, b1, W2, b2,
           ln_g, ln_b, _trace=False):
    from concourse.bass_utils import run_bass_kernel_spmd

    in_maps = _prepare_in_maps(queries, keys, values, mask, Wq, Wk, Wv, Wo,
                               W1, b1, W2, b2, ln_g, ln_b)
    nc = _get_nc()
    res = run_bass_kernel_spmd(nc, in_maps, core_ids=list(range(NC)),
                               trace=_trace)
    _CACHE["last_result"] = res

    out = np.empty((B, LQ, D), dtype=np.float32)
    for core in range(NC):
        b, half = core // 2, core % 2
        out[b, half * LQC:(half + 1) * LQC, :] = res.results[core]["yout"]
    return out
